# revision 1
# baseline (speedup 1.0000x reference)
"""Winograd F(4,3)-y variant: 6 m-terms per 4 output rows (vs F(2,3)'s
4 per 2) -> 25% fewer MACs and no y-remainder tiles (24 = 6 windows x 4).

Tiles: two z-planes x 6 wy-windows x 24 x (N=288 per k-slice; N=144
paid the full ~107ns pipe drain per MM and lost). Each k-slice gets
its OWN 1-bank PSUM tile from an 8-deep pool and is ACT-evacuated to
SBUF right after its stop matmul - WARs are per-bank and ~6 slices
stale, so the PE streams without stalling behind evacuations (a
single 6-bank tile serialized MMs behind ACT and lost, 112us).
Inverse transform (AT rows {1,1,1,1,1,0 / 0,1,-1,2,-2,0 / 0,1,1,4,4,0
/ 0,1,-1,8,-8,1}) as ACT scaled copies + DVE adds.
bf16 end-to-end rel err ~7.4e-3 (gate 2e-2).
"""

import sys

if "/opt/trn_rl_repo" not in sys.path:
    sys.path.insert(0, "/opt/trn_rl_repo")

import ml_dtypes
import numpy as np

CIN, COUT, K = 64, 128, 3
DHW = 24
ZS = 12
NP = 14
ZA, ZB0, ZB = 8, 6, 8  # chunk1 planes 0..7, chunk2 planes 6..13
PW = 26
NW = 6  # y window count (stride 4, size 6)
NK = 6
N_CORES = 8
NT = 288  # cols per k-slice (2z x 6wy x 24x)

BT = np.array(
    [
        [4, 0, -5, 0, 1, 0],
        [0, -4, -4, 1, 1, 0],
        [0, 4, -4, -1, 1, 0],
        [0, -2, -1, 2, 1, 0],
        [0, 2, -1, -2, 1, 0],
        [0, 4, 0, -5, 0, 1],
    ],
    np.float32,
)
G = np.array(
    [
        [1 / 4, 0, 0],
        [-1 / 6, -1 / 6, -1 / 6],
        [-1 / 6, 1 / 6, -1 / 6],
        [1 / 24, 1 / 12, 1 / 6],
        [1 / 24, -1 / 12, 1 / 6],
        [0, 0, 1],
    ],
    np.float32,
)

# per-k pass table: (tile, tapA, tapB); tile 0 = D (+1x), 1 = E (+1z);
# tapB None -> K=64 single on bottom half. taps are (dz, dx).
KPASSES = (
    [(0, (dz, 0), (dz, 1)) for dz in range(3)]
    + [(1, (0, 2), (1, 2))]
    + [(0, (2, 2), None)]
)
NP_K = len(KPASSES)  # 5


def _elide_redundant_ldweights(nc):
    n_drop = 0
    for f in nc.m.functions:
        for b in f.blocks:
            last_key = None
            drop = []
            for inst in b.instructions:
                if type(inst).__name__ == "InstLdweights":
                    key = (str(inst.ins[0]), str(inst.perf_mode), str(inst.is_transpose))
                    si = inst.sync_info
                    clean = si is None or (len(si.on_wait) == 0 and len(si.on_update) == 0)
                    if key == last_key and clean:
                        drop.append(inst)
                    else:
                        last_key = key
            for inst in drop:
                b.instructions.remove(inst)
            n_drop += len(drop)
    return n_drop


def _build_program(loop_n=None, unroll=False):
    import concourse.tile as tile
    from concourse import bacc, mybir

    BF16 = mybir.dt.bfloat16
    F32 = mybir.dt.float32

    nc = bacc.Bacc("TRN2")
    xins = []
    for s in "de":
        x1 = nc.declare_dram_parameter(f"x{s}1", [128, NK, ZA, NW, PW], BF16, isOutput=False)
        x2 = nc.declare_dram_parameter(f"x{s}2", [128, NK, ZB, NW, PW], BF16, isOutput=False)
        xins.append((x1, x2))
    wk_in = nc.declare_dram_parameter("wk", [128, NK * NP_K, 128], BF16, isOutput=False)
    y_out = nc.declare_dram_parameter("y", [128, ZS, DHW, DHW], BF16, isOutput=True)

    with tile.TileContext(nc) as tc:
        with (
            tc.tile_pool(name="xw", bufs=1) as xw_pool,
            tc.tile_pool(name="ps", bufs=8, space="PSUM") as ps_pool,
            tc.tile_pool(name="tmp", bufs=3) as tmp_pool,
            tc.tile_pool(name="ob", bufs=4) as ob_pool,
        ):

            def body(_iv=None):
                Wa = xw_pool.tile([128, 15, 128], BF16, name="Wa", tag="Wa")
                nc.sync.dma_start(out=Wa[:], in_=wk_in[:, 0:15])
                Wb = xw_pool.tile([128, 15, 128], BF16, name="Wb", tag="Wb")
                nc.sync.dma_start(out=Wb[:], in_=wk_in[:, 15:30])
                XT1, XT2 = [], []
                for s, (x1, x2) in zip("de", xins):
                    t1 = xw_pool.tile([128, NK, ZA, NW, PW], BF16, name=f"X{s}1", tag=f"X{s}1")
                    nc.sync.dma_start(out=t1[:], in_=x1[:])
                    XT1.append(t1)
                for s, (x1, x2) in zip("de", xins):
                    t2 = xw_pool.tile([128, NK, ZB, NW, PW], BF16, name=f"X{s}2", tag=f"X{s}2")
                    nc.sync.dma_start(out=t2[:], in_=x2[:])
                    XT2.append(t2)

                tiles = [
                    (XT1, 0, 0), (XT1, 0, 2), (XT1, 0, 4),
                    (XT2, 6, 6), (XT2, 6, 8), (XT2, 6, 10),
                ]

                for XT, zbase, zi in tiles:
                    # one 1-bank PSUM tile per k-slice from an 8-deep pool:
                    # WARs are per-bank and ~6 slices stale, so the PE never
                    # stalls behind an ACT evacuation
                    psk = [ps_pool.tile([128, 512], F32, name="ps", tag="ps") for _ in range(NK)]
                    M = tmp_pool.tile([128, NK, NT], F32, name="M", tag="M")
                    for kk in range(NK):
                        for p, (ti, ta, tb) in enumerate(KPASSES):
                            j = kk * NP_K + p
                            lo, hi = (0, 128) if tb is not None else (0, 64)
                            W = Wa if j < 15 else Wb
                            dz, dx = ta
                            zl = zi - zbase + dz
                            nc.tensor.matmul(
                                psk[kk][:, :NT],
                                lhsT=W[lo:hi, j % 15, :],
                                rhs=XT[ti][lo:hi, kk, zl : zl + 2, 0:NW, dx : dx + 24],
                                start=(p == 0),
                                stop=(p == NP_K - 1),
                                skip_group_check=True,
                            )
                        # evacuate this k-slice now: frees its PSUM bank
                        # while the remaining k's still stream
                        nc.scalar.copy(M[:, kk, :], psk[kk][:, :NT])
                    ob = ob_pool.tile([128, 2, NW, 4, 24], BF16, name="ob", tag="ob")
                    tt = {
                        nm: tmp_pool.tile([128, NT], F32, name=nm, tag=nm)
                        for nm in ("a", "b", "p", "q", "u", "q2", "p4", "q8", "v")
                    }
                    m = lambda k: M[:, k, :]
                    # AT chain: a=m1+m2 b=m1-m2 p=m3+m4 q=m3-m4;
                    # y0=m0+a+p y1=b+2q y2=a+4p y3=b+8q+m5
                    nc.vector.tensor_add(tt["a"], m(1), m(2))
                    nc.vector.tensor_sub(tt["b"], m(1), m(2))
                    nc.vector.tensor_add(tt["p"], m(3), m(4))
                    nc.vector.tensor_sub(tt["q"], m(3), m(4))
                    nc.vector.tensor_add(tt["u"], tt["a"], tt["p"])
                    nc.vector.tensor_add(ob[:, :, :, 0, :], m(0), tt["u"])
                    nc.scalar.mul(tt["q2"], tt["q"], 2.0)
                    nc.vector.tensor_add(ob[:, :, :, 1, :], tt["b"], tt["q2"])
                    nc.scalar.mul(tt["p4"], tt["p"], 4.0)
                    nc.vector.tensor_add(ob[:, :, :, 2, :], tt["a"], tt["p4"])
                    nc.scalar.mul(tt["q8"], tt["q"], 8.0)
                    nc.vector.tensor_add(tt["v"], tt["b"], tt["q8"])
                    nc.vector.tensor_add(ob[:, :, :, 3, :], m(5), tt["v"])
                    nc.sync.dma_start(out=y_out[:, zi : zi + 2, :, :], in_=ob[:])

            if loop_n is not None:
                if unroll:
                    for _k in range(loop_n):
                        body()
                else:
                    with tc.For_i(0, loop_n, 1) as _i:
                        body(_i)
            else:
                body()

    nc.finalize()
    _elide_redundant_ldweights(nc)
    return nc


def _transform_w(weight):
    w = np.asarray(weight, np.float32).reshape(COUT, CIN, K, K, K)
    gw = np.einsum("ky,oczyx->koczx", G, w)
    wk = np.zeros((128, NK * NP_K, 128), np.float32)
    for kk in range(NK):
        for p, (_ti, ta, tb) in enumerate(KPASSES):
            j = kk * NP_K + p
            wk[0:64, j, :] = gw[kk, :, :, ta[0], ta[1]].T
            if tb is not None:
                wk[64:128, j, :] = gw[kk, :, :, tb[0], tb[1]].T
    return wk.astype(ml_dtypes.bfloat16)


def _make_in_maps(x, weight):
    wk = _transform_w(weight)
    x = np.asarray(x, np.float32)
    in_maps = []
    for c in range(N_CORES):
        b, zh = divmod(c, 2)
        z0 = zh * ZS
        xpad = np.zeros((CIN, PW, PW, PW), np.float32)
        xpad[:, 1:25, 1:25, 1:25] = x[b]
        win = xpad[:, z0 : z0 + NP]  # (64, 14, 26, 26)
        # T[cin, k, z, w, x] = sum_j BT[k, j] win[cin, z, 4w+j, x]
        wmat = np.lib.stride_tricks.sliding_window_view(win, 6, axis=2)[:, :, ::4][:, :, :NW]
        # wmat: (64, 14, 6, 26, 6)
        T = np.einsum("kj,czwxj->ckzwx", BT, wmat)

        def repl(shift_axis):
            X = np.zeros((128, NK, NP, NW, PW), np.float32)
            X[0:64] = T
            if shift_axis == 4:  # +1x
                X[64:128, :, :, :, :-1] = T[:, :, :, :, 1:]
            else:  # +1z
                X[64:128, :, :-1] = T[:, :, 1:]
            return X.astype(ml_dtypes.bfloat16)

        m = {"wk": wk}
        for s, ax in (("d", 4), ("e", 0)):
            X = repl(ax)
            m[f"x{s}1"] = np.ascontiguousarray(X[:, :, 0:ZA])
            m[f"x{s}2"] = np.ascontiguousarray(X[:, :, ZB0 : ZB0 + ZB])
        in_maps.append(m)
    return in_maps


def _gather(results):
    out = np.empty((4, COUT, DHW, DHW, DHW), np.float32)
    for c in range(N_CORES):
        b, zh = divmod(c, 2)
        out[b, :, zh * ZS : (zh + 1) * ZS] = results[c]["y"].astype(np.float32)
    return out


def kernel(x, weight):
    from concourse.bass_utils import run_bass_kernel_spmd

    in_maps = _make_in_maps(x, weight)
    nc = _build_program()
    res = run_bass_kernel_spmd(nc, in_maps, list(range(N_CORES)))
    return _gather(res.results)


def _emulate_core(m):
    XT = {}
    for s in "de":
        XT[s] = (np.asarray(m[f"x{s}1"], np.float32), np.asarray(m[f"x{s}2"], np.float32))
    WK = np.asarray(m["wk"], np.float32)
    y = np.zeros((128, ZS, DHW, DHW), np.float32)
    groups = [(0, 0, [0, 2, 4]), (1, 6, [6, 8, 10])]
    bf = lambda a: a.astype(ml_dtypes.bfloat16).astype(np.float32)
    for ci, zbase, gtiles in groups:
        for zi in gtiles:
            ps = np.zeros((128, NK, NT), np.float32)
            for kk in range(NK):
                for p, (ti, ta, tb) in enumerate(KPASSES):
                    j = kk * NP_K + p
                    lo, hi = (0, 128) if tb is not None else (0, 64)
                    X = XT["de"[ti]][ci]
                    dz, dx = ta
                    zl = zi - zbase + dz
                    r = X[lo:hi, kk, zl : zl + 2, 0:NW, dx : dx + 24]
                    ps[:, kk] += WK[lo:hi, j, :].T @ r.reshape(hi - lo, -1)
            a = ps[:, 1] + ps[:, 2]
            b = ps[:, 1] - ps[:, 2]
            p_ = ps[:, 3] + ps[:, 4]
            q = ps[:, 3] - ps[:, 4]
            rows = [
                bf(ps[:, 0] + a + p_),
                bf(b + 2 * q),
                bf(a + 4 * p_),
                bf(b + 8 * q + ps[:, 5]),
            ]
            yi = np.stack(rows, 2).reshape(128, 2, NW, 24, 4).transpose(0, 1, 2, 4, 3)
            y[:, zi : zi + 2] = yi.reshape(128, 2, 24, 24)
    return y


if __name__ == "__main__":
    import jax

    sys.path.insert(0, "/root/problem")
    import reference

    cpu = jax.devices("cpu")[0]
    with jax.default_device(cpu):
        inputs = {k: np.asarray(v) for k, v in reference.setup_inputs().items()}
        expected = np.asarray(
            reference.reference(**{k: jax.device_put(v, cpu) for k, v in inputs.items()})
        )
    in_maps = _make_in_maps(inputs["x"], inputs["weight"])
    y = _emulate_core(in_maps[0])
    exp = expected[0][:, 0:ZS]
    err = np.linalg.norm(y - exp) / np.linalg.norm(exp)
    print("emulated core0 rel err:", err)

